# revision 42
# baseline (speedup 1.0000x reference)
"""Multi-head attention (B=2, S=2048, D=1024, H=16) on 8 TRN2 NeuronCores.

Sharding: batch x head-group. Core c handles batch c//4 and heads
[4*(c%4), 4*(c%4)+4). Each core computes its heads' Q/K/V projections
(column-parallel), causal attention, and a row-parallel partial of the
output projection. The host sums the 4 partials per batch (all-reduce
done on host during unshard) and adds dense_b.

On-core dataflow (everything transposed, feature-major):
  QT = WqT.T @ XqT   [256, 2048]   (fp32r matmuls, K=1024 in 8 chunks)
  KT, V likewise (V in natural [S, 256] layout, + ones column for row sums)
  per head pair, per 512-wide q-block, per 128-wide k-chunk:
    logitsT [sk=128, sq=512] = KT_chunk.T @ QT_block   (2 heads row-packed
      in the PE array at tile_position (0,0)/(64,0), shared 2-bank PSUM tile)
    diagonal chunks: add -1e9 upper-tri pattern (DVE, precomputed tile)
    PT = exp(0.125 * logitsT)                          (ScalarE, PSUM->SBUF)
    OT[65, 512] += V_aug.T @ PT    (V_aug = [V | ones] -> rows 0:64 = O^T,
                                    row 64 = softmax denominator)
  per head: recip = approx(1/sums); R[64,512] = ones x recip (K=1 matmul);
    OT_norm = OT * R; dense partial outT[., q-block] += denseT_h.T @ OT_norm
Causality is exploited by skipping fully-masked k-chunks (~2x less work).
A generic path (any mask) adds maskT.T * -8e9 to every chunk instead.
"""

import numpy as np
from contextlib import ExitStack

import concourse.tile as tile
from concourse import bacc, mybir
from concourse.bass_utils import run_bass_kernel_spmd

F32 = mybir.dt.float32
F32R = mybir.dt.float32r
AF = mybir.ActivationFunctionType
ADD = mybir.AluOpType.add
MULT = mybir.AluOpType.mult

B, S, D, H = 2, 2048, 1024, 16
NCORES = 8
HL = 4            # heads per core
DH = D // H       # 64
DLOC = HL * DH    # 256 local feature dims
SBK = 512         # seq block (q)
NSB = S // SBK    # 4
KCH = 128         # k chunk
NEG = -1e9


def _ts(i, n):
    return slice(i * n, (i + 1) * n)


def build(causal=True, with_bq=False, with_bk=False, with_bv=False, debug=False, evac='any', pt_bufs=8, iters=1, stage_o=True, late_loads=False, small_bufs=2, ot_bufs=2, ev_bufs=3):
    nc = bacc.Bacc(None, target_bir_lowering=False)

    xqT = nc.dram_tensor("xqT", [D, S], F32, kind="ExternalInput")
    xkT = nc.dram_tensor("xkT", [D, S], F32, kind="ExternalInput")
    xvT = nc.dram_tensor("xvT", [D, S], F32, kind="ExternalInput")
    wqT = nc.dram_tensor("wqT", [D, DLOC], F32, kind="ExternalInput")
    wkT = nc.dram_tensor("wkT", [D, DLOC], F32, kind="ExternalInput")
    wvT = nc.dram_tensor("wvT", [D, DLOC], F32, kind="ExternalInput")
    dnT = nc.dram_tensor("dnT", [DLOC, D], F32, kind="ExternalInput")
    if causal:
        mtri = nc.dram_tensor("mtri", [128, NSB, SBK], F32, kind="ExternalInput")
    else:
        mskT = nc.dram_tensor("mskT", [S, S], F32, kind="ExternalInput")
    bq = nc.dram_tensor("bq", [DLOC], F32, kind="ExternalInput") if with_bq else None
    bk = nc.dram_tensor("bk", [DLOC], F32, kind="ExternalInput") if with_bk else None
    bv = nc.dram_tensor("bv", [DLOC], F32, kind="ExternalInput") if with_bv else None
    outT = nc.dram_tensor("outT", [D, S], F32, kind="ExternalOutput")
    if debug:
        dQT = nc.dram_tensor("dQT", [DLOC, S], F32, kind="ExternalOutput")
        dKT = nc.dram_tensor("dKT", [DLOC, S], F32, kind="ExternalOutput")
        dV = nc.dram_tensor("dV", [128, (S // KCH) * HL * (DH + 1)], F32, kind="ExternalOutput")
        dOT = nc.dram_tensor("dOT", [NSB, HL, 64, SBK], F32, kind="ExternalOutput")
        dPT = nc.dram_tensor("dPT", [128, 2, SBK], F32, kind="ExternalOutput")
        dO = nc.dram_tensor("dO", [NSB, HL, 65, SBK], F32, kind="ExternalOutput")
        dQT2 = nc.dram_tensor("dQT2", [DLOC, SBK], F32, kind="ExternalOutput")

    ones512 = nc.inline_tensor(np.ones((1, 512), np.float32), name="ones512")
    ones128p = nc.inline_tensor(np.ones((128, 1), np.float32), name="ones128p")
    ones6464 = nc.inline_tensor(np.ones((128, 64), np.float32), name="ones6464")
    zeros128p = nc.inline_tensor(np.zeros((128, 1), np.float32), name="zeros128p")

    with tile.TileContext(nc) as tc, ExitStack() as ctx:
        pers = ctx.enter_context(tc.tile_pool(name="pers", bufs=1))
        xpool = ctx.enter_context(tc.tile_pool(name="xpool", bufs=13))
        ptp = ctx.enter_context(tc.tile_pool(name="ptp", bufs=pt_bufs))
        otp = ctx.enter_context(tc.tile_pool(name="otp", bufs=ot_bufs))
        evp = ctx.enter_context(tc.tile_pool(name="evp", bufs=ev_bufs))
        smallp = ctx.enter_context(tc.tile_pool(name="smallp", bufs=small_bufs))
        if not causal:
            mskp = ctx.enter_context(tc.tile_pool(name="mskp", bufs=3))
        mmp = ctx.enter_context(tc.tile_pool(name="mmp", bufs=2, space="PSUM"))
        lp = ctx.enter_context(tc.tile_pool(name="lp", bufs=2, space="PSUM"))
        opp = ctx.enter_context(tc.tile_pool(name="opp", bufs=1, space="PSUM"))

        # ---------- persistent loads ----------
        wparts = {}
        for wname, w_dram in (("q", wqT), ("k", wkT), ("v", wvT)):
            parts = []
            for kc2 in range(4):
                wt = pers.tile([128, 2, DLOC], F32R, tag=f"w{wname}{kc2}",
                               name=f"w_{wname}_{kc2}")
                parts.append(wt)
            wparts[wname] = parts
        dn_sb = pers.tile([128, 2, D], F32R, tag="dn")
        if not late_loads:
            nc.sync.dma_start(out=dn_sb, in_=dnT.rearrange("(c p) n -> p c n", p=128).bitcast(F32R))
        if causal:
            mtri_sb = pers.tile([128, NSB, SBK], F32, tag="mtri")
            if not late_loads:
                nc.sync.dma_start(out=mtri_sb, in_=mtri[:, :, :])

        ones_r = pers.tile([1, 512], F32R, tag="ones_r")
        nc.sync.dma_start(out=ones_r, in_=ones512[:, :].bitcast(F32R))
        onescol = pers.tile([128, 1], F32R, tag="onescol")
        nc.sync.dma_start(out=onescol, in_=ones128p[:, :].bitcast(F32R))
        ones64 = pers.tile([128, 64], F32R, tag="ones64")
        nc.sync.dma_start(out=ones64, in_=ones6464[:, :].bitcast(F32R))
        zerocol = pers.tile([128, 1], F32R, tag="zerocol")
        nc.sync.dma_start(out=zerocol, in_=zeros128p[:, :].bitcast(F32R))

        bsb = {}
        for name, dram in (("q", bq), ("k", bk), ("v", bv)):
            if dram is not None:
                t = pers.tile([1, DLOC], F32R, tag=f"b{name}")
                nc.sync.dma_start(out=t, in_=dram[None, :].bitcast(F32R))
                bsb[name] = t

        if causal:
            tri01 = pers.tile([128, KCH], F32, tag="tri01")
            nc.vector.tensor_scalar(
                out=tri01, in0=mtri_sb[:, 0, 0:KCH], scalar1=0.0, scalar2=None,
                op0=mybir.AluOpType.is_equal)
        QT_sb = pers.tile([128, 2, S], F32R, tag="QT")
        KT_sb = pers.tile([128, 2, S], F32R, tag="KT")
        V_sb = pers.tile([128, S // KCH, HL, DH + 1], F32R, tag="V")
        # ones column of V_aug (softmax denominator trick)
        nc.vector.tensor_copy(
            V_sb[:, :, :, DH:DH + 1],
            onescol[:, None, None, :].broadcast_to([128, S // KCH, HL, 1]),
        )

        outT_r = outT.rearrange("(c p) s -> p c s", p=128)

        def load_x(j, js, it=0):
            xt = {}
            for xname, src in (("q", xqT), ("k", xkT), ("v", xvT)):
                srcr = src.rearrange("(c p) s -> p c s", p=128)
                tiles = []
                for kc2 in range(4):
                    t = xpool.tile([128, 2, SBK], F32R, tag="xt",
                                   name=f"x_{xname}_{it}_{j}_{kc2}")
                    nc.sync.dma_start(
                        out=t, in_=srcr[:, _ts(kc2, 2), js].bitcast(F32R))
                    tiles.append(t)
                xt[xname] = tiles
            return xt

        def phase_A(j, js, it=0, xt=None):
            # ---------- projections for s-block j ----------
            if xt is None:
                xt = load_x(j, js, it)

            for bname, dst in (("q", QT_sb), ("k", KT_sb)):
                for mc in range(2):
                    ps = mmp.tile([128, 512], F32, tag="mm")
                    has_b = bname in bsb
                    for kc in range(8):
                        nc.tensor.matmul(
                            ps[:, :],
                            lhsT=wparts[bname][kc // 2][:, kc % 2, _ts(mc, 128)],
                            rhs=xt[bname][kc // 2][:, kc % 2, :],
                            start=(kc == 0), stop=(kc == 7 and not has_b),
                        )
                    if has_b:
                        nc.tensor.matmul(
                            ps[:, :], lhsT=bsb[bname][0:1, _ts(mc, 128)], rhs=ones_r[0:1, 0:SBK],
                            start=False, stop=True,
                        )
                    getattr(nc, evac).tensor_copy(dst[:, mc, js], ps)

            for sc in range(4):
                ps = mmp.tile([128, 512], F32, tag="mm")
                has_b = "v" in bsb
                for kc in range(8):
                    nc.tensor.matmul(
                        ps[:, 0:DLOC],
                        lhsT=xt["v"][kc // 2][:, kc % 2, _ts(sc, 128)],
                        rhs=wparts["v"][kc // 2][:, kc % 2, :],
                        start=(kc == 0), stop=(kc == 7 and not has_b),
                    )
                if has_b:
                    nc.tensor.matmul(
                        ps[:, 0:DLOC], lhsT=ones_r[0:1, 0:128], rhs=bsb["v"][0:1, :],
                        start=False, stop=True,
                    )
                getattr(nc, evac).tensor_copy(
                    V_sb[:, j * 4 + sc, :, 0:DH],
                    ps[:, 0:DLOC].rearrange("p (h d) -> p h d", h=HL),
                )

            if debug and j == 0:
                nc.sync.dma_start(
                    out=dQT2.rearrange("(c p) s -> p c s", p=128),
                    in_=QT_sb[:, :, 0:SBK].bitcast(F32),
                )

        def phase_B(j, js, it=0):
            # ---------- attention + dense for q-block j ----------
            nkc = (j + 1) * 4
            OTs = [None, None]
            for pc in range(2):
                O = [
                    opp.tile([65, 512], F32, tag=f"o{i}", name=f"O_{it}_{j}_{pc}_{i}")
                    for i in range(2)
                ]
                for kc in range(nkc) if causal else range(S // KCH):
                    L = lp.tile([128, 2, SBK], F32, tag="L")
                    for i in range(2):
                        nc.tensor.matmul(
                            L[:, i, :],
                            lhsT=KT_sb[_ts(i, 64), pc, _ts(kc, KCH)],
                            rhs=QT_sb[_ts(i, 64), pc, js],
                            start=True, stop=True,
                            tile_position=(64 * i, 0),
                        )
                    if causal:
                        pass  # mask applied multiplicatively on PT after exp
                    else:
                        mk = mskp.tile([128, SBK], F32, tag="mk")
                        nc.sync.dma_start(out=mk, in_=mskT[_ts(kc, KCH), js])
                        nc.vector.tensor_tensor(
                            out=L[:, :, :], in0=L[:, :, :],
                            in1=mk[:, None, :].broadcast_to([128, 2, SBK]),
                            op=ADD,
                        )
                    PT = ptp.tile([128, 2, SBK], F32R, tag="PT")
                    if causal and kc >= 4 * j:
                        off = (kc - 4 * j) * KCH
                        if off:
                            # zero the fully-masked left columns, exp the rest
                            nc.vector.tensor_copy(
                                PT[:, :, 0:off],
                                zerocol[:, None, :].broadcast_to([128, 2, off]),
                            )
                        nc.scalar.activation(
                            out=PT[:, :, off:SBK], in_=L[:, :, off:SBK],
                            func=AF.Exp, scale=0.125)
                        # triangle mask as 0/1 multiply (off the exp-feeding edge)
                        nc.vector.tensor_tensor(
                            out=PT[:, :, off:off + KCH],
                            in0=PT[:, :, off:off + KCH].bitcast(F32),
                            in1=tri01[:, None, :].broadcast_to([128, 2, KCH]),
                            op=MULT,
                        )
                    else:
                        nc.scalar.activation(out=PT, in_=L, func=AF.Exp, scale=0.125)
                    if debug and j == 0 and pc == 0 and kc == 0:
                        nc.sync.dma_start(out=dPT[:, :, :], in_=PT.bitcast(F32))
                    last = (kc == (nkc - 1 if causal else S // KCH - 1))
                    for i in range(2):
                        nc.tensor.matmul(
                            O[i][0:65, :],
                            lhsT=V_sb[:, kc, 2 * pc + i, :],
                            rhs=PT[:, i, :],
                            start=(kc == 0), stop=last,
                            skip_group_check=True,
                        )
                otpair = otp.tile([128, 512], F32R, tag=f"otp{pc}", name=f"otp_{it}_{j}_{pc}")
                for i in range(2):
                    h = 2 * pc + i
                    if debug:
                        ofull = evp.tile([65, 512], F32, tag="ofull")
                        nc.any.tensor_copy(ofull, O[i][0:65, :])
                        nc.sync.dma_start(out=dO[j, h], in_=ofull)
                    # softmax denominators: copy sums row (lane 64), broadcast to
                    # partitions 0:64 via K=1 matmul, reciprocal, multiply.
                    sm_sb = smallp.tile([65, 512], F32R, tag="sm")
                    nc.any.tensor_copy(sm_sb[64:65, :], O[i][64:65, :])
                    Sps = mmp.tile([128, 512], F32, tag="mm")
                    nc.tensor.matmul(
                        Sps[0:64, :], lhsT=ones64[64:65, 0:64], rhs=sm_sb[64:65, :],
                        start=True, stop=True, tile_position=(64, 0),
                    )
                    rc_sb = smallp.tile([64, 512], F32, tag="rc")
                    nc.vector.reciprocal_approx_fast(out=rc_sb, in_=Sps[0:64, :])
                    if stage_o:
                        o_in = evp.tile([64, 512], F32, tag="osb")
                        getattr(nc, evac).tensor_copy(o_in, O[i][0:64, :])
                    else:
                        o_in = O[i][0:64, :]
                    # normalize (single PSUM operand when unstaged)
                    if i == 0:
                        nc.vector.tensor_tensor(
                            out=otpair[0:64, :], in0=o_in, in1=rc_sb, op=MULT)
                        if debug:
                            nc.sync.dma_start(out=dOT[j, h], in_=otpair[0:64, :].bitcast(F32))
                    else:
                        ot_tmp = smallp.tile([64, 512], F32R, tag="ott")
                        nc.vector.tensor_tensor(out=ot_tmp, in0=o_in, in1=rc_sb, op=MULT)
                        # partition shift 0:64 -> 64:128 (SBUF->SBUF DMA)
                        nc.sync.dma_start(out=otpair[64:128, :], in_=ot_tmp[:, :])
                        if debug:
                            nc.sync.dma_start(out=dOT[j, h], in_=ot_tmp.bitcast(F32))
                OTs[pc] = otpair

            for mc in range(8):
                dps = mmp.tile([128, 512], F32, tag="mm")
                for pc in range(2):
                    nc.tensor.matmul(
                        dps[:, :], lhsT=dn_sb[:, pc, _ts(mc, 128)], rhs=OTs[pc][:, :],
                        start=(pc == 0), stop=(pc == 1),
                    )
                ev = evp.tile([128, 512], F32, tag="ev")
                getattr(nc, evac).tensor_copy(ev, dps)
                nc.sync.dma_start(out=outT_r[:, mc, js], in_=ev)

        # startup: interleave weight-part and first-block x DMAs in
        # consumption order so the first projection matmuls start ~3us in
        # (the DMA path drains in emission order at aggregate bandwidth)
        xt0 = {}
        js0 = _ts(0, SBK)
        for xname, xsrc in (("q", xqT), ("k", xkT), ("v", xvT)):
            srcr = xsrc.rearrange("(c p) s -> p c s", p=128)
            tiles = []
            wr = {"q": wqT, "k": wkT, "v": wvT}[xname].rearrange(
                "(c p) m -> p c m", p=128)
            for kc2 in range(4):
                nc.sync.dma_start(
                    out=wparts[xname][kc2],
                    in_=wr[:, _ts(kc2, 2), :].bitcast(F32R))
                t = xpool.tile([128, 2, SBK], F32R, tag="xt",
                               name=f"x_{xname}_0_0_{kc2}")
                nc.sync.dma_start(
                    out=t, in_=srcr[:, _ts(kc2, 2), js0].bitcast(F32R))
                tiles.append(t)
            xt0[xname] = tiles

        for it in range(iters):
            for j in range(NSB):
                phase_A(j, _ts(j, SBK), it, xt=(xt0 if it == 0 and j == 0 else None))
                if it == 0 and j == 0 and late_loads:
                    # deprioritized loads: queued behind the first x-block so
                    # the opening projection matmuls start sooner; still well
                    # ahead of their first consumers (diag mask in B0, dense)
                    nc.sync.dma_start(
                        out=dn_sb,
                        in_=dnT.rearrange("(c p) n -> p c n", p=128).bitcast(F32R))
                    if causal:
                        nc.sync.dma_start(out=mtri_sb, in_=mtri[:, :, :])
            for j in range(NSB):
                phase_B(j, _ts(j, SBK), it)

        if debug:
            nc.sync.dma_start(out=dQT.rearrange("(c p) s -> p c s", p=128), in_=QT_sb.bitcast(F32))
            nc.sync.dma_start(out=dKT.rearrange("(c p) s -> p c s", p=128), in_=KT_sb.bitcast(F32))
            nc.sync.dma_start(out=dV[:, :], in_=V_sb.bitcast(F32).rearrange("p a b c -> p (a b c)"))

    nc.finalize()
    return nc


_CACHE = {}


def _get_nc(causal, with_bq, with_bk, with_bv):
    key = (causal, with_bq, with_bk, with_bv)
    if key not in _CACHE:
        _CACHE[key] = build(causal, with_bq, with_bk, with_bv)
    return _CACHE[key]


def _make_mtri():
    p = np.arange(128)[:, None, None]
    o = np.arange(NSB)[None, :, None] * 128
    f = np.arange(SBK)[None, None, :]
    return np.where(p + o > f, np.float32(NEG), np.float32(0.0)).astype(np.float32)


def _prep_in_maps(query, key_, value, mask2d, causal, wq_w, wk_w, wv_w, dense_w,
                  wq_b, wk_b, wv_b, with_bq, with_bk, with_bv):
    in_maps = []
    xT = {}
    for b in range(B):
        xT[b] = (
            np.ascontiguousarray(query[b].T),
            np.ascontiguousarray(key_[b].T),
            np.ascontiguousarray(value[b].T),
        )
    mtri = _make_mtri() if causal else None
    mskT = None if causal else np.ascontiguousarray(mask2d.T * np.float32(-8e9))
    for c in range(NCORES):
        b, g = divmod(c, 4)
        sl = _ts(g, DLOC)
        m = {
            "xqT": xT[b][0], "xkT": xT[b][1], "xvT": xT[b][2],
            "wqT": np.ascontiguousarray(wq_w[sl].T),
            "wkT": np.ascontiguousarray(wk_w[sl].T),
            "wvT": np.ascontiguousarray(wv_w[sl].T),
            "dnT": np.ascontiguousarray(dense_w[:, sl].T),
        }
        if causal:
            m["mtri"] = mtri
        else:
            m["mskT"] = mskT
        if with_bq:
            m["bq"] = np.ascontiguousarray(wq_b[sl])
        if with_bk:
            m["bk"] = np.ascontiguousarray(wk_b[sl])
        if with_bv:
            m["bv"] = np.ascontiguousarray(wv_b[sl])
        in_maps.append(m)
    return in_maps


def _run(in_maps, causal, with_bq, with_bk, with_bv, **kw):
    nc = _get_nc(causal, with_bq, with_bk, with_bv)
    return run_bass_kernel_spmd(nc, in_maps, core_ids=list(range(NCORES)), **kw)


def kernel(query, key_, value, mask, wq_w, wq_b, wk_w, wk_b, wv_w, wv_b,
           dense_w, dense_b, _profile_kw=None):
    query = np.asarray(query, np.float32)
    key_ = np.asarray(key_, np.float32)
    value = np.asarray(value, np.float32)
    mask2d = np.asarray(mask, np.float32).reshape(S, S)
    wq_w = np.asarray(wq_w, np.float32)
    wk_w = np.asarray(wk_w, np.float32)
    wv_w = np.asarray(wv_w, np.float32)
    dense_w = np.asarray(dense_w, np.float32)
    wq_b = np.asarray(wq_b, np.float32)
    wk_b = np.asarray(wk_b, np.float32)
    wv_b = np.asarray(wv_b, np.float32)
    dense_b = np.asarray(dense_b, np.float32)

    causal = bool(np.array_equal(mask2d, np.triu(np.ones((S, S), np.float32), k=1)))
    with_bq = bool(np.any(wq_b))
    with_bk = bool(np.any(wk_b))
    with_bv = bool(np.any(wv_b))

    in_maps = _prep_in_maps(query, key_, value, mask2d, causal, wq_w, wk_w, wv_w,
                            dense_w, wq_b, wk_b, wv_b, with_bq, with_bk, with_bv)
    res = _run(in_maps, causal, with_bq, with_bk, with_bv, **(_profile_kw or {}))

    out = np.empty((B, S, D), np.float32)
    for b in range(B):
        acc = res.results[4 * b]["outT"].astype(np.float32).copy()
        for g in range(1, 4):
            acc += res.results[4 * b + g]["outT"]
        out[b] = acc.T + dense_b[None, :]
    if _profile_kw:
        return out, res
    return out
